# revision 5
# baseline (speedup 1.0000x reference)
"""M2 convection (SE(2) trilinear warp) Trainium2 kernel.

out[b,c,k,i,j] = x[b,c] trilinearly sampled at (theta_k, i, j) . g0[c]^{-1}.

Structure exploited: for fixed (c,k) the warp is a uniform translation.
The 8-tap trilinear blend factors into
  theta (2 taps, periodic slices a_k, a_k+1, constant weights)
  y     (2 taps, per-row integer shift, encoded exactly in a banded matrix)
  x     (2 taps, uniform free-dim shift + constant blend)

Device mapping (all heavy math in bf16; tolerance is 2e-2):
  - theta and y are folded into PE matmuls: 4 accumulating matmuls per
    (slot m, row-block u) with per-(theta-tap) scaled band matrices and
    rhs = theta slices m, m+1 of a zero-padded bf16 tile.  bf16 matmuls
    run at 1 cycle/column (fp32 is 4).
  - x 2-tap blend: one DVE scalar_tensor_tensor per slot reading the
    fp32 PSUM at offsets 0/+1, writing bf16.  The per-(c,k) integer x
    shift is resolved on the host by selecting a W-wide window from the
    slightly padded output (keeps the SPMD program shift-independent).
  - theta slice selection is made program-static by staging each
    channel's slices rotated by rot_c (a_k = k + rot_c, a bijection), so
    slot m always reads slots m, m+1.

Sharding: channels across 8 cores (2 channels/core, no communication).
"""
import sys
import numpy as np

sys.path.insert(0, "/opt/trn_rl_repo")

import concourse.mybir as mybir  # noqa: E402
from concourse import bacc, bass_utils  # noqa: E402
from concourse.tile import TileContext  # noqa: E402
from ml_dtypes import bfloat16  # noqa: E402

TWO_PI = 2.0 * np.pi
B, C, Or, H, W = 4, 16, 8, 256, 256
N_CORES = 8
C_LOC = C // N_CORES          # channels per core
N_CK = C_LOC * Or             # (c_local, slot) pairs per core

LAST_RESULTS = None


def _reference_tables(g0):
    """Replicate the reference's f32 index/weight math (jax on CPU so the
    rounding matches the jax reference bit-for-bit)."""
    import jax
    import jax.numpy as jnp

    with jax.default_device(jax.devices("cpu")[0]):
        g0 = jnp.asarray(g0, dtype=jnp.float32)
        x0, y0, th0 = g0[:, 0], g0[:, 1], g0[:, 2]
        k = jnp.arange(Or, dtype=jnp.float32)
        alpha = k[None, :] * (TWO_PI / Or) - th0[:, None]
        ca, sa = jnp.cos(alpha), jnp.sin(alpha)
        dx = ca * x0[:, None] - sa * y0[:, None]
        dy = sa * x0[:, None] + ca * y0[:, None]
        t = k[None, :] - th0[:, None] * (Or / TWO_PI)
        xs = jnp.arange(W, dtype=jnp.float32)[None, None, :] - dx[:, :, None]
        ys = jnp.arange(H, dtype=jnp.float32)[None, None, :] - dy[:, :, None]
        tf = jnp.floor(t)
        ft = t - tf
        t0i = tf.astype(jnp.int32)
        xf = jnp.floor(xs)
        fx = xs - xf
        x0i = xf.astype(jnp.int32)
        yf = jnp.floor(ys)
        fy = ys - yf
        y0i = yf.astype(jnp.int32)
        return dict(
            ft=np.asarray(ft), t0i=np.asarray(t0i),
            fx=np.asarray(fx), x0i=np.asarray(x0i),
            fy=np.asarray(fy), y0i=np.asarray(y0i),
        )


def _x_shift(tabs, c, k):
    return int(tabs["x0i"][c, k][W // 2]) - W // 2


def _pads(tabs):
    hs = [_x_shift(tabs, c, k) for c in range(C) for k in range(Or)]
    padl = -min(hs) + 2
    padr = max(hs) + 1 + 2
    return max(padl, 2), max(padr, 2)


def _core_tables(tabs, channels, padl, nu):
    """Per-core kernel inputs.

    Returns (mats, wxr, rots, hvals):
      mats [128, C_LOC, Or, 2(dt), 2(u), 2(v), 128] bf16 — band y matrices
        scaled by wt_dt * wx0 for each theta tap.
      wxr  [128, N_CK] f32 — x-blend ratio fx/(1-fx).
      rots [C_LOC] — slice rotation per channel (slot s holds slice s+rot).
      hvals[C_LOC, Or] — x shift per (channel, k); host selects window
        [padl+h, padl+h+W) from the nv-wide output.
    """
    mats = np.zeros((128, C_LOC, Or, 2, 2, 2, 128), dtype=np.float32)
    wxr = np.zeros((128, N_CK), dtype=np.float32)
    rots = np.zeros(C_LOC, dtype=np.int64)
    hvals = np.zeros((C_LOC, Or), dtype=np.int64)

    for cl, c in enumerate(channels):
        a = np.mod(tabs["t0i"][c], Or)          # [Or] first theta tap per k
        rot = int(a[0])
        assert np.all(a == (np.arange(Or) + rot) % Or), \
            f"theta map not a uniform rotation: {a}"
        rots[cl] = rot
        for k in range(Or):
            m = k                                # slot == k under rotation
            cki = cl * Or + m
            ft = np.float32(tabs["ft"][c, k])
            wt = (np.float32(1.0) - ft, ft)
            # --- x scalars (uniform-shift approximation, like reference
            # rounding noise; validated to ~1e-5) ---
            x0i = tabs["x0i"][c, k]
            fx = tabs["fx"][c, k]
            h = _x_shift(tabs, c, k)
            nonuni = np.abs(x0i - (np.arange(W) + h)).max()
            assert nonuni <= 1, f"x shift non-uniformity {nonuni} too large"
            fmid = np.float32(0.5) * (fx.min() + fx.max())
            wx0 = np.float32(1.0) - fmid
            wxr[:, cki] = fmid / wx0 if wx0 > 0 else np.float32(0)
            assert 0 <= padl + h and padl + h + 1 + W <= nu, \
                f"x shift {h} vs pads"
            hvals[cl, m] = h
            # --- y matrices (per-row exact), theta weight folded per dt ---
            y0i = tabs["y0i"][c, k]             # [H] int
            fy = tabs["fy"][c, k]               # [H] f32
            for dt in range(2):
                sc = np.float32(wt[dt]) * wx0
                for dyc in (0, 1):
                    wrow = (fy if dyc else (np.float32(1.0) - fy)) * sc
                    r = y0i + dyc                # src row per out row i
                    valid = (r >= 0) & (r < H)
                    i_idx = np.nonzero(valid)[0]
                    rv = r[i_idx]
                    mats[rv % 128, cl, m, dt, i_idx // 128, rv // 128,
                         i_idx % 128] += wrow[i_idx].astype(np.float32)
    return mats.astype(bfloat16), wxr, rots, hvals


def _build_program(padl, padr):
    nu = W + padl + padr        # padded rhs / PSUM width
    nv = nu - 1                 # output candidate width
    assert nu <= 512
    nc = bacc.Bacc("TRN2", num_devices=N_CORES)
    f32 = mybir.dt.float32
    bf16 = mybir.dt.bfloat16
    x_d = nc.dram_tensor("xs", [B, C_LOC, Or, H, W], bf16, kind="ExternalInput")
    m_d = nc.dram_tensor("mats", [128, C_LOC, Or, 2, 2, 2, 128], bf16,
                         kind="ExternalInput")
    w_d = nc.dram_tensor("wxr", [128, N_CK], f32, kind="ExternalInput")
    o_d = nc.dram_tensor("o", [B, C_LOC, Or, H, nv], bf16, kind="ExternalOutput")

    with TileContext(nc) as tc:
        with tc.tile_pool(name="const", bufs=1) as cpool, \
             tc.tile_pool(name="xin", bufs=2) as xpool, \
             tc.tile_pool(name="work", bufs=3) as wpool, \
             tc.tile_pool(name="oout", bufs=2) as opool, \
             tc.tile_pool(name="psum", bufs=4, space="PSUM") as psum:
            mt = cpool.tile([128, C_LOC, Or, 2, 2, 2, 128], bf16)
            wt = cpool.tile([128, N_CK], f32)
            nc.sync.dma_start(out=mt[:], in_=m_d.ap())
            nc.sync.dma_start(out=wt[:], in_=w_d.ap())

            for b in range(B):
                for cl in range(C_LOC):
                    x_sb = xpool.tile([128, Or, 2, nu], bf16, tag="x_sb",
                                      name="x_sb")
                    nc.scalar.memzero(x_sb[:, :, :, 0:padl])
                    nc.scalar.memzero(x_sb[:, :, :, padl + W:nu])
                    src = x_d.ap()[b, cl].rearrange("k (u p) j -> p k u j",
                                                    p=128)
                    nc.sync.dma_start(out=x_sb[:, :, :, padl:padl + W],
                                      in_=src)
                    out_sb = opool.tile([128, Or, 2, nv], bf16, tag="out_sb",
                                        name="out_sb")
                    for m in range(Or):
                        cki = cl * Or + m
                        U = psum.tile([128, 2, 512], f32, tag="U", name="U")
                        for u in range(2):
                            first, last = (0, 0), (1, 1)
                            for dt in range(2):
                                s = (m + dt) % Or
                                for v in range(2):
                                    nc.tensor.matmul(
                                        U[:, u, 0:nu],
                                        mt[:, cl, m, dt, u, v],
                                        x_sb[:, s, v, :],
                                        start=((dt, v) == first),
                                        stop=((dt, v) == last))
                        # DVE can't take two PSUM operands; stage through an
                        # ACT-engine copy (otherwise idle) with bf16 downcast.
                        A = wpool.tile([128, 2, nu], bf16, tag="A", name="A")
                        nc.scalar.copy(A[:], U[:, :, 0:nu])
                        nc.vector.scalar_tensor_tensor(
                            out=out_sb[:, m], in0=A[:, :, 1:1 + nv],
                            scalar=wt[:, cki:cki + 1], in1=A[:, :, 0:nv],
                            op0=mybir.AluOpType.mult,
                            op1=mybir.AluOpType.add)
                    dst = o_d.ap()[b, cl].rearrange("k (u p) j -> p k u j",
                                                    p=128)
                    nc.sync.dma_start(out=dst, in_=out_sb[:])
    nc.compile()
    return nc


_NC_CACHE = {}


def kernel(x, g0):
    x = np.asarray(x, dtype=np.float32)
    g0 = np.asarray(g0, dtype=np.float32)
    tabs = _reference_tables(g0)
    padl, padr = _pads(tabs)
    nu = W + padl + padr
    nv = nu - 1

    if (padl, padr) not in _NC_CACHE:
        _NC_CACHE[(padl, padr)] = _build_program(padl, padr)
    nc = _NC_CACHE[(padl, padr)]

    in_maps = []
    host_maps = []
    for core in range(N_CORES):
        channels = list(range(core * C_LOC, (core + 1) * C_LOC))
        mats, wxr, rots, hvals = _core_tables(tabs, channels, padl, nu)
        xs = np.empty((B, C_LOC, Or, H, W), dtype=bfloat16)
        for cl, c in enumerate(channels):
            perm = (np.arange(Or) + rots[cl]) % Or
            xs[:, cl] = x[:, c, perm].astype(bfloat16)
        in_maps.append({"xs": xs, "mats": mats, "wxr": wxr})
        host_maps.append(hvals)

    res = bass_utils.run_bass_kernel_spmd(nc, in_maps,
                                          core_ids=list(range(N_CORES)))
    global LAST_RESULTS
    LAST_RESULTS = res

    out = np.empty((B, C, Or, H, W), dtype=np.float32)
    for core in range(N_CORES):
        raw = res.results[core]["o"]            # [B, C_LOC, Or, H, nv] bf16
        hvals = host_maps[core]
        for cl in range(C_LOC):
            c = core * C_LOC + cl
            for k in range(Or):
                s = padl + int(hvals[cl, k])
                out[:, c, k] = raw[:, cl, k, :, s:s + W].astype(np.float32)
    return out


# revision 12
# speedup vs baseline: 1.2214x; 1.2214x over previous
"""M2 convection (SE(2) trilinear warp) Trainium2 kernel.

out[b,c,k,i,j] = x[b,c] trilinearly sampled at (theta_k, i, j) . g0[c]^{-1}.

Structure exploited: for fixed (c,k) the warp is a uniform translation.
The 8-tap trilinear blend factors into
  theta (2 taps, periodic slices a_k, a_k+1, constant weights)
  y     (2 taps, per-row integer shift, encoded exactly in a banded matrix)
  x     (2 taps, uniform free-dim shift + constant blend)

Device mapping (heavy math in bf16; tolerance is 2e-2):
  - theta and y are folded into PE matmuls.  Output rows are stored
    rolled by the per-(c,k) y shift (out' i' = (i + s_k) mod H, encoded
    in the band matrices, un-rolled on the host), which makes the band
    (almost) block-diagonal: each 128-row output block reads one 128-row
    source block, so each (slot m, block u) needs only 2 accumulating
    matmuls (theta taps dt=0,1).  The few rows whose taps straddle a
    block boundary or wrap are computed exactly on the host and
    overwritten after the device run (~4 rows per (c,k)).
  - The dt=1 matrices are the dt=0 matrices scaled by ft/(1-ft); they
    are generated on-device by a 4x-mode DVE tensor_scalar multiply to
    halve the matrix DMA.
  - x 2-tap blend: PSUM -> bf16 SBUF copy on the otherwise idle ACT/Pool
    engines, then one DVE scalar_tensor_tensor per slot.  The per-(c,k)
    integer x shift is resolved on the host by selecting a W-wide window
    from the slightly padded output (keeps the SPMD program
    shift-independent).
  - theta slice selection is program-static: each channel's slices are
    staged rotated by rot_c (a_k = k + rot_c), so slot m reads slots
    m, m+1.

Sharding: channels across 8 cores (2 channels/core, no communication).
"""
import sys
import numpy as np

sys.path.insert(0, "/opt/trn_rl_repo")

import concourse.mybir as mybir  # noqa: E402
from concourse import bacc, bass_utils  # noqa: E402
from concourse.tile import TileContext  # noqa: E402
from ml_dtypes import bfloat16  # noqa: E402

TWO_PI = 2.0 * np.pi
B, C, Or, H, W = 4, 16, 8, 256, 256
N_CORES = 8
C_LOC = C // N_CORES          # channels per core
N_CK = C_LOC * Or             # (c_local, slot) pairs per core

LAST_RESULTS = None


def _reference_tables(g0):
    """Replicate the reference's f32 index/weight math (jax on CPU so the
    rounding matches the jax reference bit-for-bit)."""
    import jax
    import jax.numpy as jnp

    with jax.default_device(jax.devices("cpu")[0]):
        g0 = jnp.asarray(g0, dtype=jnp.float32)
        x0, y0, th0 = g0[:, 0], g0[:, 1], g0[:, 2]
        k = jnp.arange(Or, dtype=jnp.float32)
        alpha = k[None, :] * (TWO_PI / Or) - th0[:, None]
        ca, sa = jnp.cos(alpha), jnp.sin(alpha)
        dx = ca * x0[:, None] - sa * y0[:, None]
        dy = sa * x0[:, None] + ca * y0[:, None]
        t = k[None, :] - th0[:, None] * (Or / TWO_PI)
        xs = jnp.arange(W, dtype=jnp.float32)[None, None, :] - dx[:, :, None]
        ys = jnp.arange(H, dtype=jnp.float32)[None, None, :] - dy[:, :, None]
        tf = jnp.floor(t)
        ft = t - tf
        t0i = tf.astype(jnp.int32)
        xf = jnp.floor(xs)
        fx = xs - xf
        x0i = xf.astype(jnp.int32)
        yf = jnp.floor(ys)
        fy = ys - yf
        y0i = yf.astype(jnp.int32)
        return dict(
            ft=np.asarray(ft), t0i=np.asarray(t0i),
            fx=np.asarray(fx), x0i=np.asarray(x0i),
            fy=np.asarray(fy), y0i=np.asarray(y0i),
        )


def _x_shift(tabs, c, k):
    return int(tabs["x0i"][c, k][W // 2]) - W // 2


def _pads(tabs):
    hs = [_x_shift(tabs, c, k) for c in range(C) for k in range(Or)]
    padl = -min(hs) + 2
    padr = max(hs) + 1 + 2
    return max(padl, 2), max(padr, 2)


def _core_tables(tabs, channels, padl, nu):
    """Per-core kernel inputs.

    Returns (mats, wxr, rth, rots, hvals, svals, patch_rows):
      mats [128, C_LOC, Or, 2(u), 128] bf16 — dt=0 band matrices, rolled
        so block u' reads only source block v=u'; scaled by (1-ft)*wx0.
      wxr  [128, N_CK] f32 — x-blend ratio fx/(1-fx).
      rth  [128, N_CK] f32 — theta ratio ft/(1-ft) (dt=1 matrix scale).
      rots [C_LOC] — slice rotation per channel.
      hvals[C_LOC, Or] — x shift (host window select).
      svals[C_LOC, Or] — y roll (host row unroll).
      patch_rows[cl][k] — out rows to recompute exactly on the host.
    """
    mats = np.zeros((128, C_LOC, Or, 2, 128), dtype=np.float32)
    wxr = np.zeros((128, N_CK), dtype=np.float32)
    rth = np.zeros((128, N_CK), dtype=np.float32)
    rots = np.zeros(C_LOC, dtype=np.int64)
    hvals = np.zeros((C_LOC, Or), dtype=np.int64)
    svals = np.zeros((C_LOC, Or), dtype=np.int64)
    patch_rows = [[None] * Or for _ in range(C_LOC)]

    for cl, c in enumerate(channels):
        a = np.mod(tabs["t0i"][c], Or)          # [Or] first theta tap per k
        rot = int(a[0])
        assert np.all(a == (np.arange(Or) + rot) % Or), \
            f"theta map not a uniform rotation: {a}"
        rots[cl] = rot
        for k in range(Or):
            m = k                                # slot == k under rotation
            cki = cl * Or + m
            ft = np.float32(tabs["ft"][c, k])
            wt0 = np.float32(1.0) - ft
            assert wt0 > 1e-5, f"degenerate theta weight wt0={wt0}"
            rth[:, cki] = ft / wt0
            # --- x scalars (uniform-shift approximation; the residual
            # rounding non-uniformity is ~1e-5) ---
            x0i = tabs["x0i"][c, k]
            fx = tabs["fx"][c, k]
            h = _x_shift(tabs, c, k)
            nonuni = np.abs(x0i - (np.arange(W) + h)).max()
            assert nonuni <= 1, f"x shift non-uniformity {nonuni} too large"
            fmid = np.float32(0.5) * (fx.min() + fx.max())
            wx0 = np.float32(1.0) - fmid
            wxr[:, cki] = fmid / wx0 if wx0 > 0 else np.float32(0)
            assert 0 <= padl + h and padl + h + 1 + W <= nu, \
                f"x shift {h} vs pads"
            hvals[cl, m] = h
            # --- y matrices, rolled by sk ---
            y0i = tabs["y0i"][c, k]             # [H] int
            fy = tabs["fy"][c, k]               # [H] f32
            ii = np.arange(H)
            sk = int((y0i - ii).min())
            assert (y0i - ii).max() - sk <= 1, "y shift non-uniformity"
            svals[cl, m] = sk
            ip = ii + sk                         # un-modded rolled row
            # rows handled on the host: wrapped rows with any valid tap,
            # and rows whose taps straddle the 128-row block boundary
            patch = np.zeros(H, dtype=bool)
            for dyc in (0, 1):
                r = y0i + dyc
                valid = (r >= 0) & (r < H)
                wrapped = (ip < 0) | (ip >= H)
                straddle = valid & ~wrapped & ((r // 128) != (ip // 128))
                patch |= (wrapped & valid) | straddle
                d = r - ip
                assert np.all((d[valid] >= 0) & (d[valid] <= 2)), \
                    f"band offset out of range: {np.unique(d[valid])}"
            patch_rows[cl][m] = np.nonzero(patch)[0]
            sc = wt0 * wx0
            for dyc in (0, 1):
                wrow = (fy if dyc else (np.float32(1.0) - fy)) * sc
                r = y0i + dyc
                sel = ((r >= 0) & (r < H) & ~patch
                       & (ip >= 0) & (ip < H))
                i_idx = np.nonzero(sel)[0]
                rv = r[i_idx]
                ipv = ip[i_idx]
                assert np.all(rv // 128 == ipv // 128)
                mats[rv % 128, cl, m, ipv // 128, ipv % 128] += \
                    wrow[i_idx].astype(np.float32)
    return (mats.astype(bfloat16), wxr, rth, rots, hvals, svals,
            patch_rows)


def _patch_exact(x, tabs, c, k, rows):
    """Exact (f32-table) 8-tap warp for the given out rows: [B, len, W]."""
    if len(rows) == 0:
        return None
    ft = np.float64(tabs["ft"][c, k])
    a0 = int(np.mod(tabs["t0i"][c, k], Or))
    y0i = tabs["y0i"][c, k][rows]
    fy = tabs["fy"][c, k][rows].astype(np.float64)
    x0i = tabs["x0i"][c, k]
    fx = tabs["fx"][c, k].astype(np.float64)
    out = np.zeros((B, len(rows), W), dtype=np.float64)
    for dt in (0, 1):
        wt = ft if dt else (1.0 - ft)
        sl = x[:, c, (a0 + dt) % Or].astype(np.float64)  # [B, H, W]
        for dyc in (0, 1):
            r = y0i + dyc
            wy = (fy if dyc else (1.0 - fy)) * ((r >= 0) & (r < H))
            rc = np.clip(r, 0, H - 1)
            for dxc in (0, 1):
                xi = x0i + dxc
                wx = (fx if dxc else (1.0 - fx)) * ((xi >= 0) & (xi < W))
                xic = np.clip(xi, 0, W - 1)
                out += (wt * wy[None, :, None] * wx[None, None, :]
                        * sl[:, rc][:, :, xic])
    return out.astype(np.float32)


def _build_program(padl, padr):
    nu = W + padl + padr        # padded rhs / PSUM width
    nv = nu - 1                 # output candidate width
    assert nu <= 512
    nc = bacc.Bacc("TRN2", num_devices=N_CORES)
    f32 = mybir.dt.float32
    bf16 = mybir.dt.bfloat16
    x_d = nc.dram_tensor("xs", [B, C_LOC, Or, H, W], bf16, kind="ExternalInput")
    m_d = nc.dram_tensor("mats", [128, C_LOC, Or, 2, 128], bf16,
                         kind="ExternalInput")
    w_d = nc.dram_tensor("wxr", [128, N_CK], f32, kind="ExternalInput")
    r_d = nc.dram_tensor("rth", [128, N_CK], f32, kind="ExternalInput")
    o_d = nc.dram_tensor("o", [B, C_LOC, Or, H, nv], bf16, kind="ExternalOutput")

    with TileContext(nc) as tc:
        with tc.tile_pool(name="const", bufs=1) as cpool, \
             tc.tile_pool(name="xin", bufs=2) as xpool, \
             tc.tile_pool(name="work", bufs=3) as wpool, \
             tc.tile_pool(name="oout", bufs=2) as opool, \
             tc.tile_pool(name="psum", bufs=4, space="PSUM") as psum:
            # mt holds both theta-tap matrix sets; dt=1 is generated on
            # device from the DMA'd dt=0 set (4x-mode DVE multiply).
            mt = cpool.tile([128, 2, C_LOC, Or, 2, 128], bf16)
            wt = cpool.tile([128, N_CK], f32)
            rt = cpool.tile([128, N_CK], f32)

            def load_mats(cl):
                nc.sync.dma_start(out=mt[:, 0, cl], in_=m_d.ap()[:, cl])

            def gen_dt1(cl):
                for m in range(Or):
                    nc.vector.tensor_scalar_mul(
                        mt[:, 1, cl, m], mt[:, 0, cl, m],
                        rt[:, cl * Or + m:cl * Or + m + 1])

            # DMA-queue order: first unit's deps first, the rest in the
            # shadow of its compute.
            nc.sync.dma_start(out=wt[:], in_=w_d.ap())
            nc.sync.dma_start(out=rt[:], in_=r_d.ap())
            load_mats(0)

            for b in range(B):
                for cl in range(C_LOC):
                    x_sb = xpool.tile([128, Or, 2, nu], bf16, tag="x_sb",
                                      name="x_sb")
                    nc.scalar.memzero(x_sb[:, :, :, 0:padl])
                    nc.scalar.memzero(x_sb[:, :, :, padl + W:nu])
                    src = x_d.ap()[b, cl].rearrange("k (u p) j -> p k u j",
                                                    p=128)
                    for sq in range(0, Or, 4):
                        nc.sync.dma_start(
                            out=x_sb[:, sq:sq + 4, :, padl:padl + W],
                            in_=src[:, sq:sq + 4])
                    if b == 0 and cl == 0:
                        load_mats(1)
                        gen_dt1(0)
                        gen_dt1(1)
                    out_sb = opool.tile([128, Or, 2, nv], bf16, tag="out_sb",
                                        name="out_sb")
                    for m in range(Or):
                        cki = cl * Or + m
                        U = psum.tile([128, 2, 512], f32, tag="U", name="U")
                        for u in range(2):
                            for dt in range(2):
                                nc.tensor.matmul(
                                    U[:, u, 0:nu],
                                    mt[:, dt, cl, m, u],
                                    x_sb[:, (m + dt) % Or, u, :],
                                    start=(dt == 0), stop=(dt == 1))
                        # DVE can't take two PSUM operands and GPSIMD can't
                        # read PSUM at all; stage through a bf16 downcast
                        # copy on the otherwise idle ACT engine.
                        A = wpool.tile([128, 2, nu], bf16, tag="A", name="A")
                        nc.scalar.copy(A[:], U[:, :, 0:nu])
                        nc.vector.scalar_tensor_tensor(
                            out=out_sb[:, m], in0=A[:, :, 1:1 + nv],
                            scalar=wt[:, cki:cki + 1], in1=A[:, :, 0:nv],
                            op0=mybir.AluOpType.mult,
                            op1=mybir.AluOpType.add)
                    dst = o_d.ap()[b, cl].rearrange("k (u p) j -> p k u j",
                                                    p=128)
                    for mq in range(0, Or, 2):
                        nc.sync.dma_start(out=dst[:, mq:mq + 2],
                                          in_=out_sb[:, mq:mq + 2])
    nc.compile()
    return nc


_NC_CACHE = {}


def kernel(x, g0):
    x = np.asarray(x, dtype=np.float32)
    g0 = np.asarray(g0, dtype=np.float32)
    tabs = _reference_tables(g0)
    padl, padr = _pads(tabs)
    nu = W + padl + padr

    if (padl, padr) not in _NC_CACHE:
        _NC_CACHE[(padl, padr)] = _build_program(padl, padr)
    nc = _NC_CACHE[(padl, padr)]

    in_maps = []
    host_maps = []
    for core in range(N_CORES):
        channels = list(range(core * C_LOC, (core + 1) * C_LOC))
        mats, wxr, rth, rots, hvals, svals, patch_rows = \
            _core_tables(tabs, channels, padl, nu)
        xs = np.empty((B, C_LOC, Or, H, W), dtype=bfloat16)
        for cl, c in enumerate(channels):
            perm = (np.arange(Or) + rots[cl]) % Or
            xs[:, cl] = x[:, c, perm].astype(bfloat16)
        in_maps.append({"xs": xs, "mats": mats, "wxr": wxr, "rth": rth})
        host_maps.append((hvals, svals, patch_rows))

    res = bass_utils.run_bass_kernel_spmd(nc, in_maps,
                                          core_ids=list(range(N_CORES)))
    global LAST_RESULTS
    LAST_RESULTS = res

    out = np.empty((B, C, Or, H, W), dtype=np.float32)
    for core in range(N_CORES):
        raw = res.results[core]["o"]            # [B, C_LOC, Or, H, nv] bf16
        hvals, svals, patch_rows = host_maps[core]
        for cl in range(C_LOC):
            c = core * C_LOC + cl
            for k in range(Or):
                s = padl + int(hvals[cl, k])
                win = raw[:, cl, k, :, s:s + W].astype(np.float32)
                out[:, c, k] = np.roll(win, -int(svals[cl, k]), axis=1)
                rows = patch_rows[cl][k]
                if len(rows):
                    out[:, c, k, rows] = _patch_exact(x, tabs, c, k, rows)
    return out
